# revision 1
# baseline (speedup 1.0000x reference)
"""Trainium2 Bass kernel for nn_Attention (B=16, N=1024, C=1024, H=16, pre-LN +
q/k post-LN attention block), data-parallel over 8 NeuronCores (2 batches/core).

Per core (batch shard [2, 1024, 1024]), processed one batch at a time so q/k/v
never leave SBUF (no DRAM staging, no small-row DMA descriptors):
  A1: y = LN(x) over C; PE-transpose y -> yT [c, tok].
  A2: qT/kT = W'c @ yT (weights host-pre-centered per head: post-LN mean
      subtraction folds into them; the 1/8 attention scale folds into q's
      rstd epsilon form); per-head rstd via squares + ones-matmul partition
      reduction, broadcast back via a PE matmul; v in natural [tok, d]
      layout with a ones column (softmax denominator).
  B:  per head-pair: S^T = kT^T qT (2 heads packed in PE row groups),
      E = exp(S^T) on ScalarE (safe: |logits| <= 8 after the q/k LNs),
      O_aug^T = [V|1]^T E accumulated on PE (row 64 = denominator),
      normalize via DVE reciprocal + PE partition-broadcast -> AO^T.
  C:  out = AO^T^T @ Wp^T + bp (bias via K=1 ones matmul), fp32 out.

All matmuls run in float32r (11-bit mantissa, ~2.4e-4 rel err, bf16-rate on
PE). Weights are streamed per batch in 2 MB half-tiles (double-buffered).
"""

import numpy as np

B, N, C, H, Dh = 16, 1024, 1024, 16, 64
NCORES = 8
BL = B // NCORES          # batches per core
T = BL * N                # tokens per core
CCH = C // 128            # contraction chunks
EPS = 1e-6

_cache: dict = {}


def _round_f32r(x: np.ndarray) -> np.ndarray:
    """Round fp32 to float32r (11-bit mantissa, round-to-nearest-even)."""
    b = np.ascontiguousarray(x, dtype=np.float32).view(np.uint32)
    s = np.uint32(12)
    r = ((b + np.uint32((1 << 11) - 1) + ((b >> s) & np.uint32(1))) >> s) << s
    return r.view(np.float32).copy()


def _build(phases="full"):
    from contextlib import ExitStack

    import concourse.bacc as bacc
    import concourse.mybir as mybir
    import concourse.tile as tile

    F32 = mybir.dt.float32
    F32R = mybir.dt.float32r
    AF = mybir.ActivationFunctionType
    OP = mybir.AluOpType

    nc = bacc.Bacc("TRN2", target_bir_lowering=False, debug=False, num_devices=NCORES)

    x_d = nc.dram_tensor("x", [T, C], F32, kind="ExternalInput").ap()
    wqt_d = nc.dram_tensor("wqt", [C, C], F32R, kind="ExternalInput").ap()
    wkt_d = nc.dram_tensor("wkt", [C, C], F32R, kind="ExternalInput").ap()
    wvt_d = nc.dram_tensor("wvt", [C, C], F32R, kind="ExternalInput").ap()
    wpt_d = nc.dram_tensor("wpt", [C, C], F32R, kind="ExternalInput").ap()
    bp_d = nc.dram_tensor("bp", [1, C], F32R, kind="ExternalInput").ap()
    out_d = nc.dram_tensor("out", [T, C], F32, kind="ExternalOutput").ap()

    ce2_d = nc.dram_tensor("c_e2", [128, 2], F32R, kind="ExternalInput").ap()
    cb2_d = nc.dram_tensor("c_b2", [2, 128], F32R, kind="ExternalInput").ap()
    cones_d = nc.dram_tensor("c_ones", [128, 128], F32R, kind="ExternalInput").ap()
    cident_d = nc.dram_tensor("c_ident", [128, 128], F32R, kind="ExternalInput").ap()
    ceps_d = nc.dram_tensor("c_eps", [128, 2], F32, kind="ExternalInput").ap()

    NB = N // 128            # 8 token tiles per batch
    with tile.TileContext(nc) as tc, ExitStack() as top:
        const = top.enter_context(tc.tile_pool(name="const", bufs=1))
        ident = const.tile([128, 128], F32R)
        nc.sync.dma_start(out=ident, in_=cident_d)
        e2 = const.tile([128, 2], F32R)
        nc.sync.dma_start(out=e2, in_=ce2_d)
        b2 = const.tile([2, 128], F32R)
        nc.sync.dma_start(out=b2, in_=cb2_d)
        cones = const.tile([128, 128], F32R)
        nc.sync.dma_start(out=cones, in_=cones_d)
        onesc = cones[:, 0:64]
        ones1 = cones[0:1, :]
        ceps = const.tile([128, 2], F32)
        nc.sync.dma_start(out=ceps, in_=ceps_d)
        eps_t = ceps[:, 0:1]
        eps64_t = ceps[:, 1:2]
        bp_sb = const.tile([1, C], F32R)
        nc.sync.dma_start(out=bp_sb, in_=bp_d)

        for b in range(BL):
            with ExitStack() as bctx:
                ytp = bctx.enter_context(tc.tile_pool(name=f"yt{b}", bufs=1))
                yT = ytp.tile([128, CCH, N], F32R)

                # ---- A1: pre-LN + transpose ----
                with ExitStack() as ph:
                    a1 = ph.enter_context(tc.tile_pool(name="a1", bufs=3))
                    a1s = ph.enter_context(tc.tile_pool(name="a1s", bufs=4))
                    a1ps = ph.enter_context(
                        tc.tile_pool(name="a1ps", bufs=4, space="PSUM"))
                    nsub = max(1, C // nc.vector.BN_STATS_FMAX)
                    for t in range(NB):
                        r0 = b * N + t * 128
                        xt = a1.tile([128, C], F32, tag="xt")
                        nc.sync.dma_start(out=xt, in_=x_d[r0:r0 + 128, :])
                        stats = a1s.tile(
                            [128, nsub, nc.vector.BN_STATS_DIM], F32, tag="st")
                        xg = xt.rearrange("p (s f) -> p s f", s=nsub)
                        for s in range(nsub):
                            nc.vector.bn_stats(out=stats[:, s, :], in_=xg[:, s, :])
                        mv = a1s.tile([128, nc.vector.BN_AGGR_DIM], F32, tag="mv")
                        nc.vector.bn_aggr(out=mv, in_=stats)
                        std = a1s.tile([128, 1], F32, tag="sd")
                        nc.scalar.activation(std, mv[:, 1:2], AF.Sqrt, bias=eps_t)
                        rstd = a1s.tile([128, 1], F32, tag="rs")
                        nc.vector.reciprocal(rstd, std)
                        y = a1.tile([128, C], F32R, tag="y")
                        nc.vector.tensor_scalar(
                            out=y, in0=xt, scalar1=mv[:, 0:1], scalar2=rstd,
                            op0=OP.subtract, op1=OP.mult)
                        for cc in range(CCH):
                            tp = a1ps.tile([128, 128], F32R, tag="tp")
                            nc.tensor.transpose(
                                tp, y[:, cc * 128:(cc + 1) * 128], ident)
                            nc.vector.tensor_copy(
                                out=yT[:, cc, t * 128:(t + 1) * 128], in_=tp)

                qkv = bctx.enter_context(tc.tile_pool(name=f"qkv{b}", bufs=1))
                qT = qkv.tile([128, CCH, N], F32R)
                kT = qkv.tile([128, CCH, N], F32R)
                vS = qkv.tile([128, NB, H, Dh + 1], F32R)

                # ---- A2: projections (weights streamed in half-tiles) ----
                with ExitStack() as ph:
                    wpool = ph.enter_context(tc.tile_pool(name="wpool", bufs=2))
                    wk = ph.enter_context(tc.tile_pool(name="wk", bufs=3))
                    pp = ph.enter_context(tc.tile_pool(name="pp", bufs=3, space="PSUM"))
                    sp = ph.enter_context(tc.tile_pool(name="sp", bufs=2, space="PSUM"))
                    bcp = ph.enter_context(
                        tc.tile_pool(name="bcp", bufs=2, space="PSUM"))

                    for wi, (w_dram, o_big) in enumerate(
                            [(wqt_d, qT), (wkt_d, kT)]):
                        wh = [wpool.tile([128, CCH // 2, C], F32R, tag="w",
                                         name=f"wh{i}")
                              for i in range(2)]
                        wr = w_dram.rearrange("(h cc p) d -> h p cc d", h=2, p=128)
                        nc.sync.dma_start(out=wh[0], in_=wr[0])
                        nc.sync.dma_start(out=wh[1], in_=wr[1])
                        for dc in range(8):
                            for t2 in range(2):
                                ps = pp.tile([128, 512], F32, tag="pp")
                                for cc in range(CCH):
                                    nc.tensor.matmul(
                                        ps,
                                        wh[cc // 4][:, cc % 4,
                                                    dc * 128:(dc + 1) * 128],
                                        yT[:, cc, t2 * 512:(t2 + 1) * 512],
                                        start=(cc == 0), stop=(cc == CCH - 1))
                                qraw = wk.tile([128, 512], F32R, tag="qraw")
                                nc.vector.tensor_copy(out=qraw, in_=ps)
                                sq = wk.tile([128, 512], F32R, tag="sq")
                                nc.vector.tensor_mul(sq, qraw, qraw)
                                ssq = sp.tile([2, 512], F32, tag="ss")
                                nc.tensor.matmul(ssq, e2, sq, start=True, stop=True)
                                stdt = wk.tile([2, 512], F32, tag="stdt")
                                if wi == 0:
                                    # 0.125/sqrt(ssq/64+eps) = 1/sqrt(ssq+64eps)
                                    nc.scalar.activation(
                                        stdt, ssq, AF.Sqrt, bias=eps64_t[0:2, :])
                                else:
                                    nc.scalar.activation(
                                        stdt, ssq, AF.Sqrt,
                                        bias=eps_t[0:2, :], scale=1.0 / 64.0)
                                rst = wk.tile([2, 512], F32R, tag="rst")
                                with nc.allow_low_precision(reason="f32r rstd"):
                                    nc.vector.reciprocal(rst, stdt)
                                bc = bcp.tile([128, 512], F32, tag="bc")
                                nc.tensor.matmul(bc, b2, rst, start=True, stop=True)
                                nc.vector.tensor_mul(
                                    o_big[:, dc, t2 * 512:(t2 + 1) * 512],
                                    qraw, bc)

                    # v projection + ones column
                    wh = [wpool.tile([128, CCH // 2, C], F32R, tag="w", name=f"wh{i}")
                          for i in range(2)]
                    wr = wvt_d.rearrange("(h cc p) d -> h p cc d", h=2, p=128)
                    nc.sync.dma_start(out=wh[0], in_=wr[0])
                    nc.sync.dma_start(out=wh[1], in_=wr[1])
                    for tt in range(NB):
                        for d2 in range(2):
                            ps = pp.tile([128, 512], F32, tag="pp")
                            for cc in range(CCH):
                                nc.tensor.matmul(
                                    ps,
                                    yT[:, cc, tt * 128:(tt + 1) * 128],
                                    wh[cc // 4][:, cc % 4,
                                                d2 * 512:(d2 + 1) * 512],
                                    start=(cc == 0), stop=(cc == CCH - 1))
                            nc.vector.tensor_copy(
                                out=vS[:, tt, d2 * 8:(d2 + 1) * 8, 0:64],
                                in_=ps.rearrange("p (h e) -> p h e", e=64))
                        nc.vector.tensor_copy(
                            out=vS[:, tt, :, 64:65],
                            in_=cones[:, 0:H].rearrange("p (h e) -> p h e", e=1))

                # ---- B: attention ----
                aop = bctx.enter_context(tc.tile_pool(name=f"ao{b}", bufs=1))
                AO = aop.tile([128, CCH, N], F32R, tag="AO")
                with ExitStack() as ph:
                    be = ph.enter_context(tc.tile_pool(name="be", bufs=4))
                    bo = ph.enter_context(tc.tile_pool(name="bo", bufs=4))
                    stp = ph.enter_context(
                        tc.tile_pool(name="stp", bufs=4, space="PSUM"))
                    oap = ph.enter_context(
                        tc.tile_pool(name="oap", bufs=3, space="PSUM"))
                    bc2 = ph.enter_context(
                        tc.tile_pool(name="bc2", bufs=1, space="PSUM"))
                    for hp in range(H // 2):
                        for qc in range(2):
                            o0 = oap.tile([65, 512], F32, tag="oa")
                            o1 = oap.tile([65, 512], F32, tag="oa")
                            for kc in range(NB):
                                s0 = stp.tile([128, 512], F32, tag="st")
                                nc.tensor.matmul(
                                    s0,
                                    kT[0:64, hp, kc * 128:(kc + 1) * 128],
                                    qT[0:64, hp, qc * 512:(qc + 1) * 512],
                                    start=True, stop=True)
                                e0 = be.tile([128, 512], F32R, tag="e")
                                nc.scalar.activation(e0, s0, AF.Exp)
                                nc.tensor.matmul(
                                    o0, vS[:, kc, 2 * hp, :], e0,
                                    start=(kc == 0), stop=(kc == NB - 1))
                                s1 = stp.tile([128, 512], F32, tag="st")
                                nc.tensor.matmul(
                                    s1,
                                    kT[64:128, hp, kc * 128:(kc + 1) * 128],
                                    qT[64:128, hp, qc * 512:(qc + 1) * 512],
                                    start=True, stop=True)
                                e1 = be.tile([128, 512], F32R, tag="e")
                                nc.scalar.activation(e1, s1, AF.Exp)
                                nc.tensor.matmul(
                                    o1, vS[:, kc, 2 * hp + 1, :], e1,
                                    start=(kc == 0), stop=(kc == NB - 1))
                            for par, oo in ((0, o0), (1, o1)):
                                ao_slice = AO[par * 64:par * 64 + 64, hp,
                                              qc * 512:(qc + 1) * 512]
                                r = bo.tile([128, 512], F32R, tag="r")
                                with nc.allow_low_precision(reason="f32r recip"):
                                    nc.vector.reciprocal(r[64:65, :], oo[64:65, :])
                                bc = bc2.tile([64, 512], F32, tag="bc2")
                                nc.tensor.matmul(
                                    bc, onesc[64:65, :], r[64:65, :],
                                    start=True, stop=True, tile_position=(64, 0))
                                osb = bo.tile([64, 512], F32, tag="osb")
                                nc.vector.tensor_copy(out=osb, in_=oo[0:64, :])
                                if par == 0:
                                    nc.vector.tensor_mul(ao_slice, osb, bc)
                                else:
                                    tmp2 = bo.tile([64, 512], F32R, tag="tmp2")
                                    nc.vector.tensor_mul(tmp2, osb, bc)
                                    nc.sync.dma_start(out=ao_slice, in_=tmp2)

                # ---- C: output projection ----
                with ExitStack() as ph:
                    wpool = ph.enter_context(tc.tile_pool(name="wpc", bufs=2))
                    co = ph.enter_context(tc.tile_pool(name="co", bufs=3))
                    cps = ph.enter_context(
                        tc.tile_pool(name="cps", bufs=4, space="PSUM"))
                    wh = [wpool.tile([128, CCH // 2, C], F32R, tag="w", name=f"wh{i}")
                          for i in range(2)]
                    wr = wpt_d.rearrange("(h cc p) d -> h p cc d", h=2, p=128)
                    nc.sync.dma_start(out=wh[0], in_=wr[0])
                    nc.sync.dma_start(out=wh[1], in_=wr[1])
                    for tt in range(NB):
                        for d2 in range(2):
                            ps = cps.tile([128, 512], F32, tag="cp")
                            nc.tensor.matmul(
                                ps, ones1, bp_sb[:, d2 * 512:(d2 + 1) * 512],
                                start=True, stop=False)
                            for cc in range(CCH):
                                nc.tensor.matmul(
                                    ps,
                                    AO[:, cc, tt * 128:(tt + 1) * 128],
                                    wh[cc // 4][:, cc % 4,
                                                d2 * 512:(d2 + 1) * 512],
                                    start=False, stop=(cc == CCH - 1))
                            o_sb = co.tile([128, 512], F32, tag="osb")
                            nc.vector.tensor_copy(out=o_sb, in_=ps)
                            nc.sync.dma_start(
                                out=out_d[b * N + tt * 128:b * N + (tt + 1) * 128,
                                          d2 * 512:(d2 + 1) * 512],
                                in_=o_sb)

    nc.compile()
    return nc


def _get_nc():
    if "nc" not in _cache:
        _cache["nc"] = _build()
    return _cache["nc"]


def _host_inputs(Wq, Wk, Wv, Wp, bp):
    """Shared (core-independent) derived weight tensors."""
    def center(Wm):
        Wh = Wm.reshape(H, Dh, C)
        return (Wh - Wh.mean(axis=1, keepdims=True)).reshape(C, C)

    e2 = np.zeros((128, 2), np.float32)
    e2[0:64, 0] = 1.0
    e2[64:128, 1] = 1.0
    b2 = np.zeros((2, 128), np.float32)
    b2[0, 0:64] = 1.0
    b2[1, 64:128] = 1.0
    eps = np.zeros((128, 2), np.float32)
    eps[:, 0] = EPS
    eps[:, 1] = 64.0 * EPS
    return {
        "c_e2": e2,
        "c_b2": b2,
        "c_ones": np.ones((128, 128), np.float32),
        "c_ident": np.eye(128, dtype=np.float32),
        "c_eps": eps,
        "wqt": _round_f32r(np.ascontiguousarray(center(np.asarray(Wq)).T)),
        "wkt": _round_f32r(np.ascontiguousarray(center(np.asarray(Wk)).T)),
        "wvt": _round_f32r(np.ascontiguousarray(np.asarray(Wv).T)),
        "wpt": _round_f32r(np.ascontiguousarray(np.asarray(Wp).T)),
        "bp": _round_f32r(np.asarray(bp).reshape(1, C)),
    }


def kernel(x, Wq, Wk, Wv, Wp, bp):
    from concourse.bass_utils import run_bass_kernel_spmd

    nc = _get_nc()
    shared = _host_inputs(Wq, Wk, Wv, Wp, bp)
    x = np.asarray(x, dtype=np.float32)
    in_maps = [
        dict(shared, x=np.ascontiguousarray(x[c * BL:(c + 1) * BL].reshape(T, C)))
        for c in range(NCORES)
    ]
    res = run_bass_kernel_spmd(nc, in_maps, core_ids=list(range(NCORES)))
    out = np.stack([res.results[c]["out"].reshape(BL, N, C)
                    for c in range(NCORES)])
    return out.reshape(B, N, C).astype(np.float32)

